# revision 9
# baseline (speedup 1.0000x reference)
"""Trainium2 Bass kernel for nn_CausalPatternDetector.

Computes mean |corr(x[1:, i], x[:-1, j])| over i != j for x [32768, 1024] f32.

Strategy (8 NeuronCores, data-parallel over time T):
  - Core k gets rows x[4096k : 4096k+4097] (core 7 zero-padded by one row).
  - Each core computes its partial cross-product Xc^T @ Xl ([1024,1024],
    contraction 4096) on the TensorEngine in bf16 (full-rate), plus exact
    fp32 partial per-feature sums / sums-of-squares (DVE/ACT, hidden under
    the matmuls; partition-reduced with tiny fp32 ones-matmuls).
  - One ReduceScatter sums the [F,F]+stats partials and hands each core a
    128-row shard; each core normalizes its shard (|cov|/sqrt(nc*nl)),
    masks the diagonal, reduces to a partial scalar.
  - Host sums the 8 scalars.
"""

import numpy as np

import concourse.bass as bass
import concourse.mybir as mybir
import concourse.tile as tile
from concourse import bacc
from concourse.bass_utils import run_bass_kernel_spmd

P = 128
F = 1024
T = 32768
TS = 4096            # t-values per core
KT = TS // P         # 32 K-tiles per core
NCORES = 8
NT = float(T - 1)    # 32767 normalization count
F32 = mybir.dt.float32
BF16 = mybir.dt.bfloat16
SHARD_ROWS = P + 4   # 128 cross rows + [S_l, q_l, S_c_loc, q_c_loc]

_CACHE = {}


def _build():
    nc = bacc.Bacc("TRN2", target_bir_lowering=False, debug=False,
                   num_devices=NCORES)

    xb = nc.dram_tensor("xb", [TS + 1, F], F32, kind="ExternalInput")
    fixr = nc.dram_tensor("fix", [1, F], F32, kind="ExternalInput")
    dmask = nc.dram_tensor("dmask", [P, F], F32, kind="ExternalInput")
    out = nc.dram_tensor("out", [1, 1], F32, kind="ExternalOutput")

    add = mybir.AluOpType.add

    with tile.TileContext(nc) as tc:
        with (
            tc.tile_pool(name="dram", bufs=1, space="DRAM") as dram,
            tc.tile_pool(name="stgp", bufs=4) as stgp,
            tc.tile_pool(name="xlp", bufs=1) as xlp,
            tc.tile_pool(name="xcp", bufs=4) as xcp,
            tc.tile_pool(name="xcr", bufs=4) as xcr,
            tc.tile_pool(name="sqp", bufs=2) as sqp,
            tc.tile_pool(name="outp", bufs=2) as outp,
            tc.tile_pool(name="statp", bufs=1) as statp,
            tc.tile_pool(name="smallp", bufs=1) as smallp,
            tc.tile_pool(name="p2p", bufs=1) as p2p,
            tc.tile_pool(name="psum", bufs=8, space="PSUM") as psum,
        ):
            rs_in = dram.tile([NCORES * SHARD_ROWS, F], F32)
            rs_out = dram.tile([SHARD_ROWS, F], F32)

            # ---- stream x block in; keep bf16-rounded Xl tiles resident,
            #      accumulate exact fp32 stats from the fp32 staging ----
            sum_acc = statp.tile([P, F], F32)
            sq_acc = statp.tile([P, F], F32)
            xl = []
            for k in range(KT):
                stg = stgp.tile([P, F], F32, name=f"stg{k}", tag="stg")
                nc.sync.dma_start(stg[:], xb[P * k: P * k + P, :])
                xlk = xlp.tile([P, F], BF16, name=f"xl{k}", tag=f"xl{k}")
                nc.vector.tensor_copy(xlk[:], stg[:])
                xl.append(xlk)
                if k == 0:
                    nc.vector.tensor_copy(sum_acc[:], stg[:])
                    nc.scalar.square(sq_acc[:], stg[:])
                else:
                    sq_t = sqp.tile([P, F], F32, tag="sqt")
                    nc.scalar.square(sq_t[:], stg[:])
                    nc.vector.tensor_add(sum_acc[:], sum_acc[:], stg[:])
                    nc.vector.tensor_add(sq_acc[:], sq_acc[:], sq_t[:])

            # ---- main matmul: cross = Xc^T @ Xl, two groups of 4 m-chunks
            #      (4m x 2n = 8 PSUM banks per group) ----
            for mg in range(2):
                pst = [[psum.tile([P, 512], F32, name=f"ps_{mg}_{mi}_{n}",
                                  tag="ps")
                        for n in range(2)] for mi in range(4)]
                for k in range(KT):
                    xc_t = xcp.tile([P, 512], F32, name=f"xc_{mg}_{k}",
                                    tag="xc")
                    nc.sync.dma_start(
                        xc_t[:],
                        xb[P * k + 1: P * k + P + 1,
                           512 * mg: 512 * mg + 512])
                    xc_b = xcr.tile([P, 512], BF16, name=f"xcb_{mg}_{k}",
                                    tag="xcb")
                    nc.scalar.copy(xc_b[:], xc_t[:])
                    for mi in range(4):
                        lhsT = xc_b[:, P * mi: P * mi + P]
                        for n in range(2):
                            nc.tensor.matmul(
                                pst[mi][n][:],
                                lhsT,
                                xl[k][:, 512 * n: 512 * n + 512],
                                start=(k == 0),
                                stop=(k == KT - 1),
                            )
                for mi in range(4):
                    m = mg * 4 + mi          # shard index == m-chunk index
                    ot = outp.tile([P, F], F32, name=f"ot{m}", tag="ot")
                    for n in range(2):
                        nc.vector.tensor_copy(
                            ot[:, 512 * n: 512 * n + 512], pst[mi][n][:])
                    nc.sync.dma_start(
                        rs_in[SHARD_ROWS * m: SHARD_ROWS * m + P, :], ot[:])

            # ---- finalize per-core stats (tiny fp32 matmuls) ----
            ones = smallp.tile([P, 1], F32)
            nc.vector.memset(ones[:], 1.0)
            srow = smallp.tile([8, F], F32)   # row 0: raw col-sums of Xl rows
            qrow = smallp.tile([8, F], F32)   # row 0: raw col-sums of Xl^2
            for half in range(2):
                sl = slice(512 * half, 512 * half + 512)
                ps_a = psum.tile([1, 512], F32, name=f"psa{half}", tag="ps")
                nc.tensor.matmul(ps_a[:], ones[:], sum_acc[:, sl],
                                 start=True, stop=True)
                nc.vector.tensor_copy(srow[0:1, sl], ps_a[:])
                ps_b = psum.tile([1, 512], F32, name=f"psb{half}", tag="ps")
                nc.tensor.matmul(ps_b[:], ones[:], sq_acc[:, sl],
                                 start=True, stop=True)
                nc.vector.tensor_copy(qrow[0:1, sl], ps_b[:])

            # boundary rows: r0 = xb[0], rT = xb[4096], fix (core7: x[32767])
            r0 = smallp.tile([8, F], F32)
            nc.sync.dma_start(r0[0:1, :], xb[0:1, :])
            rT = smallp.tile([8, F], F32)
            nc.sync.dma_start(rT[0:1, :], xb[TS:TS + 1, :])
            fx = smallp.tile([8, F], F32)
            nc.sync.dma_start(fx[0:1, :], fixr[:])

            # S_c = srow - r0 + rT ; q_c = qrow - r0^2 + rT^2
            tmpr = smallp.tile([8, F], F32)
            s_c = smallp.tile([8, F], F32)
            nc.vector.tensor_sub(s_c[0:1, :], srow[0:1, :], r0[0:1, :])
            nc.vector.tensor_add(s_c[0:1, :], s_c[0:1, :], rT[0:1, :])
            q_c = smallp.tile([8, F], F32)
            nc.scalar.square(tmpr[0:1, :], r0[0:1, :])
            nc.vector.tensor_sub(q_c[0:1, :], qrow[0:1, :], tmpr[0:1, :])
            nc.scalar.square(tmpr[0:1, :], rT[0:1, :])
            nc.vector.tensor_add(q_c[0:1, :], q_c[0:1, :], tmpr[0:1, :])
            # then in place: srow -> S_l = srow - fix ; qrow -> q_l
            nc.scalar.square(tmpr[0:1, :], fx[0:1, :])
            nc.vector.tensor_sub(qrow[0:1, :], qrow[0:1, :], tmpr[0:1, :])
            nc.vector.tensor_sub(srow[0:1, :], srow[0:1, :], fx[0:1, :])
            s_l = srow
            q_l = qrow

            # write stat rows into every shard of rs_in. Row layout per
            # shard s: +128 S_l (full), +129 q_l (full), +130 S_c chunk s in
            # cols 0:128, +131 q_c chunk s in cols 0:128. Cols 128:F of the
            # +130/+131 rows get valid-but-unused filler so the RS never
            # sums uninitialized DRAM.
            for s in range(NCORES):
                base = SHARD_ROWS * s
                nc.sync.dma_start(rs_in[base + 128: base + 129, :],
                                  s_l[0:1, :])
                nc.sync.dma_start(rs_in[base + 129: base + 130, :],
                                  q_l[0:1, :])
                nc.sync.dma_start(rs_in[base + 130: base + 131, 0:P],
                                  s_c[0:1, P * s: P * s + P])
                nc.sync.dma_start(rs_in[base + 130: base + 131, P:F],
                                  s_c[0:1, 0:F - P])
                nc.sync.dma_start(rs_in[base + 131: base + 132, 0:P],
                                  q_c[0:1, P * s: P * s + P])
                nc.sync.dma_start(rs_in[base + 131: base + 132, P:F],
                                  q_c[0:1, 0:F - P])

            # ---- reduce-scatter the partial statistics ----
            nc.gpsimd.collective_compute(
                "ReduceScatter",
                add,
                replica_groups=[list(range(NCORES))],
                ins=[rs_in.opt()],
                outs=[rs_out.opt()],
            )

            # ---- phase 2: normalize this core's 128-row shard ----
            cr = p2p.tile([P, F], F32)
            nc.sync.dma_start(cr[:], rs_out[0:P, :])
            st = p2p.tile([4, F], F32)
            nc.sync.dma_start(st[:], rs_out[P:P + 4, :])
            msk = p2p.tile([P, F], F32)
            nc.sync.dma_start(msk[:], dmask[:])

            # separate partition-0 copies of the stat rows: ACT/DVE cannot
            # read from a nonzero base partition, so row views of `st`
            # (partitions 1..3) are only legal as matmul rhs.
            rsl = p2p.tile([4, F], F32)
            nc.sync.dma_start(rsl[0:1, :], rs_out[P: P + 1, :])
            rql = p2p.tile([4, F], F32)
            nc.sync.dma_start(rql[0:1, :], rs_out[P + 1: P + 2, :])
            rscl = p2p.tile([4, F], F32)
            nc.sync.dma_start(rscl[0:1, :], rs_out[P + 2: P + 3, :])
            rqcl = p2p.tile([4, F], F32)
            nc.sync.dma_start(rqcl[0:1, :], rs_out[P + 3: P + 4, :])

            g_sl = rsl[0:1, :]         # summed S_l [1, F]
            g_ql = rql[0:1, :]         # summed q_l [1, F]
            g_scl = rscl[0:1, 0:P]     # summed S_c[128s:128s+128] [1, 128]
            g_qcl = rqcl[0:1, 0:P]

            inv_n = 1.0 / NT
            rsqrt_n = float(np.sqrt(inv_n))

            # nl[j] = q_l[j] - S_l[j]^2/N  (row 0 of a zeroed [4, F] tile;
            # rows 1-3 ride along as rhs rows so they must be finite)
            nl8 = p2p.tile([4, F], F32)
            nc.vector.memset(nl8[:], 0.0)
            nc.scalar.activation(nl8[0:1, :], g_sl,
                                 mybir.ActivationFunctionType.Square,
                                 scale=rsqrt_n)
            nc.vector.tensor_sub(nl8[0:1, :], g_ql, nl8[0:1, :])

            # ncol row [1,128]: q_c - S_c^2/N ; scn row [1,128] = S_c/N.
            # Row 0 of zeroed [4,128] tiles -> K=4 lhsT for the rank-1
            # outer products (zero rows kill the other rhs rows).
            sc8 = p2p.tile([4, P], F32)
            nc.vector.memset(sc8[:], 0.0)
            nc.scalar.mul(sc8[0:1, :], g_scl, inv_n)
            nc8 = p2p.tile([4, P], F32)
            nc.vector.memset(nc8[:], 0.0)
            nc.scalar.activation(nc8[0:1, :], g_scl,
                                 mybir.ActivationFunctionType.Square,
                                 scale=rsqrt_n)
            nc.vector.tensor_sub(nc8[0:1, :], g_qcl, nc8[0:1, :])

            # cov = cr - outer(S_c/N, S_l); dsq = outer(ncol, nl)
            work = p2p.tile([P, F], F32)
            for half in range(2):
                sl = slice(512 * half, 512 * half + 512)
                po = psum.tile([P, 512], F32, name=f"po{half}", tag="ps")
                nc.tensor.matmul(po[:], sc8[:], st[0:4, sl],
                                 start=True, stop=True)
                nc.vector.tensor_sub(cr[:, sl], cr[:, sl], po[:])
                pd = psum.tile([P, 512], F32, name=f"pd{half}", tag="ps")
                nc.tensor.matmul(pd[:], nc8[:], nl8[:, sl],
                                 start=True, stop=True)
                nc.vector.tensor_copy(work[:, sl], pd[:])

            # work = 1/sqrt(dsq); then |cov| * work * mask, reduced
            nc.vector.reciprocal(work[:], work[:])
            nc.scalar.sqrt(work[:], work[:])
            nc.vector.tensor_mul(work[:], work[:], cr[:])
            nc.vector.tensor_mul(work[:], work[:], msk[:])
            rsum = p2p.tile([P, 1], F32)
            nc.vector.tensor_reduce(rsum[:], work[:], mybir.AxisListType.X,
                                    add, apply_absolute_value=True)

            fin = psum.tile([1, 1], F32, name="fin", tag="ps")
            nc.tensor.matmul(fin[:], ones[:], rsum[:], start=True, stop=True)
            fout = smallp.tile([1, 1], F32)
            nc.scalar.mul(fout[:], fin[:], 1.0 / (F * (F - 1.0)))
            nc.sync.dma_start(out[:], fout[:])

    nc.compile()
    return nc


def _in_maps(x: np.ndarray):
    x = np.ascontiguousarray(x, dtype=np.float32)
    maps = []
    for k in range(NCORES):
        lo = TS * k
        hi = min(lo + TS + 1, T)
        xb = np.zeros((TS + 1, F), dtype=np.float32)
        xb[: hi - lo] = x[lo:hi]
        fix = np.zeros((1, F), dtype=np.float32)
        if k == NCORES - 1:
            fix[0] = x[T - 1]
        dmask = np.ones((P, F), dtype=np.float32)
        dmask[np.arange(P), P * k + np.arange(P)] = 0.0
        maps.append({"xb": xb, "fix": fix, "dmask": dmask})
    return maps


def kernel(x: np.ndarray, _trace: bool = False, **_):
    if "nc" not in _CACHE:
        _CACHE["nc"] = _build()
    nc = _CACHE["nc"]
    res = run_bass_kernel_spmd(nc, _in_maps(x), core_ids=list(range(NCORES)),
                               trace=_trace)
    total = np.float32(0.0)
    for k in range(NCORES):
        total += np.float32(res.results[k]["out"][0, 0])
    _CACHE["last_results"] = res
    return np.asarray(total, dtype=np.float32)


# revision 11
# speedup vs baseline: 1.0765x; 1.0765x over previous
"""Trainium2 Bass kernel for nn_CausalPatternDetector.

Computes mean |corr(x[1:, i], x[:-1, j])| over i != j for x [32768, 1024] f32.

Strategy (8 NeuronCores, data-parallel over time T):
  - Core k gets rows x[4096k : 4096k+4097] (core 7 zero-padded by one row).
  - Each core computes its partial cross-product Xc^T @ Xl ([1024,1024],
    contraction 4096) on the TensorEngine in bf16 (full-rate), plus exact
    fp32 partial per-feature sums / sums-of-squares (DVE/ACT, hidden under
    the matmuls; partition-reduced with tiny fp32 ones-matmuls that also
    replicate the result across 8 partitions).
  - One ReduceScatter sums the [F,F]+stats partials and hands each core a
    128-row shard; each core normalizes its shard (|cov|/sqrt(nc*nl)),
    masks the diagonal, reduces to a partial scalar.
  - Host sums the 8 scalars.
"""

import numpy as np

import concourse.bass as bass
import concourse.mybir as mybir
import concourse.tile as tile
from concourse import bacc
from concourse.bass_utils import run_bass_kernel_spmd

P = 128
F = 1024
T = 32768
TS = 4096            # t-values per core
KT = TS // P         # 32 K-tiles per core
NCORES = 8
NT = float(T - 1)    # 32767 normalization count
F32 = mybir.dt.float32
BF16 = mybir.dt.bfloat16
SHARD_ROWS = P + 4   # 128 cross rows + [S_l, q_l, S_c_loc, q_c_loc]

_CACHE = {}


def _build():
    nc = bacc.Bacc("TRN2", target_bir_lowering=False, debug=False,
                   num_devices=NCORES)

    xb = nc.dram_tensor("xb", [TS + 1, F], F32, kind="ExternalInput")
    fixr = nc.dram_tensor("fix", [1, F], F32, kind="ExternalInput")
    dmask = nc.dram_tensor("dmask", [P, F], F32, kind="ExternalInput")
    out = nc.dram_tensor("out", [1, 1], F32, kind="ExternalOutput")

    add = mybir.AluOpType.add
    AF = mybir.ActivationFunctionType

    with tile.TileContext(nc) as tc:
        with (
            tc.tile_pool(name="dram", bufs=1, space="DRAM") as dram,
            tc.tile_pool(name="stgp", bufs=4) as stgp,
            tc.tile_pool(name="xlp", bufs=1) as xlp,
            tc.tile_pool(name="xcp", bufs=4) as xcp,
            tc.tile_pool(name="xcr", bufs=4) as xcr,
            tc.tile_pool(name="sqp", bufs=2) as sqp,
            tc.tile_pool(name="outp", bufs=2) as outp,
            tc.tile_pool(name="statp", bufs=1) as statp,
            tc.tile_pool(name="smallp", bufs=1) as smallp,
            tc.tile_pool(name="p2p", bufs=1) as p2p,
            tc.tile_pool(name="psum", bufs=8, space="PSUM") as psum,
        ):
            rs_in = dram.tile([NCORES * SHARD_ROWS, F], F32)
            rs_out = dram.tile([SHARD_ROWS, F], F32)
            rsv = rs_in.rearrange("(s r) f -> s r f", r=SHARD_ROWS)

            # early small loads (gpsimd queue, off the bulk sync queue)
            msk = p2p.tile([P, F], F32)
            nc.gpsimd.dma_start(msk[:], dmask[:])
            r08 = smallp.tile([8, F], F32)
            nc.gpsimd.dma_start(r08[:], xb[0:1, :].to_broadcast((8, F)))
            rT8 = smallp.tile([8, F], F32)
            nc.gpsimd.dma_start(rT8[:], xb[TS:TS + 1, :].to_broadcast((8, F)))
            fx8 = smallp.tile([8, F], F32)
            nc.gpsimd.dma_start(fx8[:], fixr[0:1, :].to_broadcast((8, F)))
            ones8 = smallp.tile([P, 8], F32)
            nc.vector.memset(ones8[:], 1.0)
            zfill = smallp.tile([8, F - P], F32)
            nc.vector.memset(zfill[:], 0.0)

            sum_acc = statp.tile([P, F], F32)
            sq_acc = statp.tile([P, F], F32)

            def psum_group(mg):
                return [[psum.tile([P, 512], F32, name=f"ps_{mg}_{mi}_{n}",
                                   tag="ps")
                         for n in range(2)] for mi in range(4)]

            def xc_load(mg, k):
                xc_t = xcp.tile([P, 512], F32, name=f"xc_{mg}_{k}", tag="xc")
                nc.sync.dma_start(
                    xc_t[:],
                    xb[P * k + 1: P * k + P + 1, 512 * mg: 512 * mg + 512])
                xc_b = xcr.tile([P, 512], BF16, name=f"xcb_{mg}_{k}",
                                tag="xcb")
                nc.scalar.copy(xc_b[:], xc_t[:])
                return xc_b

            def mm_k(pst, xc_b, xlk, k):
                for mi in range(4):
                    lhsT = xc_b[:, P * mi: P * mi + P]
                    for n in range(2):
                        nc.tensor.matmul(
                            pst[mi][n][:], lhsT,
                            xlk[:, 512 * n: 512 * n + 512],
                            start=(k == 0), stop=(k == KT - 1))

            def flush_group(mg, pst):
                for mi in range(4):
                    m = mg * 4 + mi      # shard index == m-chunk index
                    ot = outp.tile([P, F], F32, name=f"ot{m}", tag="ot")
                    for n in range(2):
                        nc.vector.tensor_copy(
                            ot[:, 512 * n: 512 * n + 512], pst[mi][n][:])
                    nc.sync.dma_start(rsv[m, 0:P, :], ot[:])

            # ---- mg0: interleave x-block streaming, bf16 casts, stats
            #      accumulation, and the first half of the matmuls ----
            xl = []
            pst0 = psum_group(0)
            for k in range(KT):
                stg = stgp.tile([P, F], F32, name=f"stg{k}", tag="stg")
                nc.sync.dma_start(stg[:], xb[P * k: P * k + P, :])
                xlk = xlp.tile([P, F], BF16, name=f"xl{k}", tag=f"xl{k}")
                nc.vector.tensor_copy(xlk[:], stg[:])
                xl.append(xlk)
                xc_b = xc_load(0, k)
                mm_k(pst0, xc_b, xlk, k)
                if k == 0:
                    nc.vector.tensor_copy(sum_acc[:], stg[:])
                    nc.scalar.square(sq_acc[:], stg[:])
                else:
                    sq_t = sqp.tile([P, F], F32, tag="sqt")
                    nc.scalar.square(sq_t[:], stg[:])
                    nc.vector.tensor_add(sum_acc[:], sum_acc[:], stg[:])
                    nc.vector.tensor_add(sq_acc[:], sq_acc[:], sq_t[:])
            flush_group(0, pst0)

            # ---- stats finalize: ones[128,8] matmul partition-reduces AND
            #      replicates the sums across 8 partitions; runs between
            #      the two matmul groups so it overlaps mg1 ----
            srow8 = smallp.tile([8, F], F32)
            qrow8 = smallp.tile([8, F], F32)
            for half in range(2):
                sl = slice(512 * half, 512 * half + 512)
                ps_a = psum.tile([8, 512], F32, name=f"psa{half}", tag="ps")
                nc.tensor.matmul(ps_a[:], ones8[:], sum_acc[:, sl],
                                 start=True, stop=True)
                nc.vector.tensor_copy(srow8[:, sl], ps_a[:])
                ps_b = psum.tile([8, 512], F32, name=f"psb{half}", tag="ps")
                nc.tensor.matmul(ps_b[:], ones8[:], sq_acc[:, sl],
                                 start=True, stop=True)
                nc.vector.tensor_copy(qrow8[:, sl], ps_b[:])

            # S_c = srow - r0 + rT ; q_c = qrow - r0^2 + rT^2   (all [8,F])
            tmp8 = smallp.tile([8, F], F32)
            s_c8 = smallp.tile([8, F], F32)
            nc.vector.tensor_sub(s_c8[:], srow8[:], r08[:])
            nc.vector.tensor_add(s_c8[:], s_c8[:], rT8[:])
            q_c8 = smallp.tile([8, F], F32)
            nc.scalar.square(tmp8[:], r08[:])
            nc.vector.tensor_sub(q_c8[:], qrow8[:], tmp8[:])
            nc.scalar.square(tmp8[:], rT8[:])
            nc.vector.tensor_add(q_c8[:], q_c8[:], tmp8[:])
            # in place: srow8 -> S_l, qrow8 -> q_l
            nc.scalar.square(tmp8[:], fx8[:])
            nc.vector.tensor_sub(qrow8[:], qrow8[:], tmp8[:])
            nc.vector.tensor_sub(srow8[:], srow8[:], fx8[:])

            # stat writes: 6 DMAs total (gpsimd queue).
            # rows +128/+129: full S_l/q_l per shard (partition s -> shard s)
            nc.gpsimd.dma_start(rsv[:, 128, :], srow8[:])
            nc.gpsimd.dma_start(rsv[:, 129, :], qrow8[:])
            # rows +130/+131: S_c/q_c chunk s in cols 0:128 (one row source
            # splits into 8 consecutive 128-wide chunks), zero filler after
            nc.gpsimd.dma_start(rsv[:, 130, 0:P], s_c8[0:1, :])
            nc.gpsimd.dma_start(rsv[:, 131, 0:P], q_c8[0:1, :])
            nc.gpsimd.dma_start(rsv[:, 130, P:F], zfill[:])
            nc.gpsimd.dma_start(rsv[:, 131, P:F], zfill[:])

            # ---- mg1 ----
            pst1 = psum_group(1)
            for k in range(KT):
                xc_b = xc_load(1, k)
                mm_k(pst1, xc_b, xl[k], k)
            flush_group(1, pst1)

            # ---- reduce-scatter the partial statistics ----
            nc.gpsimd.collective_compute(
                "ReduceScatter",
                add,
                replica_groups=[list(range(NCORES))],
                ins=[rs_in.opt()],
                outs=[rs_out.opt()],
            )

            # ---- phase 2: normalize this core's 128-row shard ----
            cr = p2p.tile([P, F], F32)
            nc.sync.dma_start(cr[:], rs_out[0:P, :])
            st = p2p.tile([4, F], F32)
            nc.scalar.dma_start(st[:], rs_out[P:P + 4, :])
            # partition-0 copies of the stat rows (ACT/DVE cannot read from
            # a nonzero base partition; row views of `st` are matmul-only)
            rsl = p2p.tile([4, F], F32)
            nc.scalar.dma_start(rsl[0:1, :], rs_out[P: P + 1, :])
            rql = p2p.tile([4, F], F32)
            nc.gpsimd.dma_start(rql[0:1, :], rs_out[P + 1: P + 2, :])
            rscl = p2p.tile([4, F], F32)
            nc.gpsimd.dma_start(rscl[0:1, :], rs_out[P + 2: P + 3, :])
            rqcl = p2p.tile([4, F], F32)
            nc.scalar.dma_start(rqcl[0:1, :], rs_out[P + 3: P + 4, :])

            g_sl = rsl[0:1, :]         # summed S_l [1, F]
            g_ql = rql[0:1, :]         # summed q_l [1, F]
            g_scl = rscl[0:1, 0:P]     # summed S_c[128s:128s+128] [1, 128]
            g_qcl = rqcl[0:1, 0:P]

            inv_n = 1.0 / NT
            rsqrt_n = float(np.sqrt(inv_n))

            # nl[j] = q_l[j] - S_l[j]^2/N  (row 0 of a zeroed [4, F] tile;
            # rows 1-3 ride along as rhs rows so they must be finite)
            nl8 = p2p.tile([4, F], F32)
            nc.vector.memset(nl8[:], 0.0)
            nc.scalar.activation(nl8[0:1, :], g_sl, AF.Square, scale=rsqrt_n)
            nc.vector.tensor_sub(nl8[0:1, :], g_ql, nl8[0:1, :])

            # ncol row [1,128]: q_c - S_c^2/N ; scn row [1,128] = S_c/N.
            # Row 0 of zeroed [4,128] tiles -> K=4 lhsT for the rank-1
            # outer products (zero rows kill the other rhs rows).
            sc8 = p2p.tile([4, P], F32)
            nc.vector.memset(sc8[:], 0.0)
            nc.scalar.mul(sc8[0:1, :], g_scl, inv_n)
            nc8 = p2p.tile([4, P], F32)
            nc.vector.memset(nc8[:], 0.0)
            nc.scalar.activation(nc8[0:1, :], g_scl, AF.Square, scale=rsqrt_n)
            nc.vector.tensor_sub(nc8[0:1, :], g_qcl, nc8[0:1, :])

            # cov = cr - outer(S_c/N, S_l); dsq = outer(ncol, nl)
            work = p2p.tile([P, F], F32)
            for half in range(2):
                sl = slice(512 * half, 512 * half + 512)
                po = psum.tile([P, 512], F32, name=f"po{half}", tag="ps")
                nc.tensor.matmul(po[:], sc8[:], st[0:4, sl],
                                 start=True, stop=True)
                nc.vector.tensor_sub(cr[:, sl], cr[:, sl], po[:])
                pd = psum.tile([P, 512], F32, name=f"pd{half}", tag="ps")
                nc.tensor.matmul(pd[:], nc8[:], nl8[:, sl],
                                 start=True, stop=True)
                nc.vector.tensor_copy(work[:, sl], pd[:])

            # work = 1/sqrt(dsq) in one ACT op; |cov| * work * mask, reduced
            nc.scalar.activation(work[:], work[:], AF.Abs_reciprocal_sqrt)
            nc.vector.tensor_mul(work[:], work[:], cr[:])
            nc.vector.tensor_mul(work[:], work[:], msk[:])
            rsum = p2p.tile([P, 1], F32)
            nc.vector.tensor_reduce(rsum[:], work[:], mybir.AxisListType.X,
                                    add, apply_absolute_value=True)

            fin = psum.tile([1, 1], F32, name="fin", tag="ps")
            nc.tensor.matmul(fin[:], ones8[:, 0:1], rsum[:],
                             start=True, stop=True)
            fout = smallp.tile([1, 1], F32)
            nc.scalar.mul(fout[:], fin[:], 1.0 / (F * (F - 1.0)))
            nc.sync.dma_start(out[:], fout[:])

    nc.compile()
    return nc


def _in_maps(x: np.ndarray):
    x = np.ascontiguousarray(x, dtype=np.float32)
    maps = []
    for k in range(NCORES):
        lo = TS * k
        hi = min(lo + TS + 1, T)
        xb = np.zeros((TS + 1, F), dtype=np.float32)
        xb[: hi - lo] = x[lo:hi]
        fix = np.zeros((1, F), dtype=np.float32)
        if k == NCORES - 1:
            fix[0] = x[T - 1]
        dmask = np.ones((P, F), dtype=np.float32)
        dmask[np.arange(P), P * k + np.arange(P)] = 0.0
        maps.append({"xb": xb, "fix": fix, "dmask": dmask})
    return maps


def kernel(x: np.ndarray, _trace: bool = False, **_):
    if "nc" not in _CACHE:
        _CACHE["nc"] = _build()
    nc = _CACHE["nc"]
    res = run_bass_kernel_spmd(nc, _in_maps(x), core_ids=list(range(NCORES)),
                               trace=_trace)
    total = np.float32(0.0)
    for k in range(NCORES):
        total += np.float32(res.results[k]["out"][0, 0])
    _CACHE["last_results"] = res
    return np.asarray(total, dtype=np.float32)


# revision 12
# speedup vs baseline: 1.2477x; 1.1590x over previous
"""Trainium2 Bass kernel for nn_CausalPatternDetector.

Computes mean |corr(x[1:, i], x[:-1, j])| over i != j for x [32768, 1024] f32.

Strategy (8 NeuronCores, data-parallel over time T):
  - Core k gets rows x[4096k : 4096k+4097] (core 7 zero-padded by one row).
  - Each core computes its partial cross-product Xc^T @ Xl ([1024,1024],
    contraction 4096) on the TensorEngine in bf16 (full-rate), plus exact
    fp32 partial per-feature sums / sums-of-squares (DVE/ACT, hidden under
    the matmuls; partition-reduced with tiny fp32 ones-matmuls that also
    replicate the result across 8 partitions).
  - Two bf16 ReduceScatters sum the [F,F]+stats partials: RS_A carries
    cross rows 0:512 (ready after the first matmul group, so it overlaps
    the second) plus the stat rows; RS_B carries rows 512:1024. Core k
    ends up owning cross rows {64k..64k+64} u {512+64k..512+64k+64}.
  - Each core normalizes its 128 rows (|cov|/sqrt(nc*nl)), masks the
    diagonal, reduces to a partial scalar. Host sums the 8 scalars.
"""

import numpy as np

import concourse.bass as bass
import concourse.mybir as mybir
import concourse.tile as tile
from concourse import bacc
from concourse.bass_utils import run_bass_kernel_spmd

P = 128
F = 1024
T = 32768
TS = 4096            # t-values per core
KT = TS // P         # 32 K-tiles per core
NCORES = 8
NT = float(T - 1)    # 32767 normalization count
F32 = mybir.dt.float32
BF16 = mybir.dt.bfloat16
RA = 68              # RS_A piece rows: 64 cross + S_l + q_l + ScA + qcA
RB = 66              # RS_B piece rows: 64 cross + ScB + qcB

_CACHE = {}


def _build():
    nc = bacc.Bacc("TRN2", target_bir_lowering=False, debug=False,
                   num_devices=NCORES)

    xb = nc.dram_tensor("xb", [TS + 1, F], F32, kind="ExternalInput")
    fixr = nc.dram_tensor("fix", [1, F], F32, kind="ExternalInput")
    dmask = nc.dram_tensor("dmask", [P, F], F32, kind="ExternalInput")
    out = nc.dram_tensor("out", [1, 1], F32, kind="ExternalOutput")

    add = mybir.AluOpType.add
    AF = mybir.ActivationFunctionType

    with tile.TileContext(nc) as tc:
        with (
            tc.tile_pool(name="dram", bufs=1, space="DRAM") as dram,
            tc.tile_pool(name="stgp", bufs=4) as stgp,
            tc.tile_pool(name="xlp", bufs=1) as xlp,
            tc.tile_pool(name="xcp", bufs=4) as xcp,
            tc.tile_pool(name="xcr", bufs=4) as xcr,
            tc.tile_pool(name="sqp", bufs=2) as sqp,
            tc.tile_pool(name="outp", bufs=2) as outp,
            tc.tile_pool(name="statp", bufs=1) as statp,
            tc.tile_pool(name="smallp", bufs=1) as smallp,
            tc.tile_pool(name="p2p", bufs=1) as p2p,
            tc.tile_pool(name="psum", bufs=8, space="PSUM") as psum,
        ):
            rsA_in = dram.tile([NCORES * RA, F], BF16)
            rsA_out = dram.tile([RA, F], BF16)
            rsB_in = dram.tile([NCORES * RB, F], BF16)
            rsB_out = dram.tile([RB, F], BF16)
            rsvA = rsA_in.rearrange("(p r) f -> p r f", r=RA)
            rsvB = rsB_in.rearrange("(p r) f -> p r f", r=RB)

            sum_acc = statp.tile([P, F], F32)
            sq_acc = statp.tile([P, F], F32)

            def psum_group(mg):
                return [[psum.tile([P, 512], F32, name=f"ps_{mg}_{mi}_{n}",
                                   tag="ps")
                         for n in range(2)] for mi in range(4)]

            def xc_load(mg, k):
                xc_t = xcp.tile([P, 512], F32, name=f"xc_{mg}_{k}", tag="xc")
                nc.sync.dma_start(
                    xc_t[:],
                    xb[P * k + 1: P * k + P + 1, 512 * mg: 512 * mg + 512])
                xc_b = xcr.tile([P, 512], BF16, name=f"xcb_{mg}_{k}",
                                tag="xcb")
                nc.scalar.copy(xc_b[:], xc_t[:])
                return xc_b

            def mm_k(pst, xc_b, xlk, k):
                for mi in range(4):
                    lhsT = xc_b[:, P * mi: P * mi + P]
                    for n in range(2):
                        nc.tensor.matmul(
                            pst[mi][n][:], lhsT,
                            xlk[:, 512 * n: 512 * n + 512],
                            start=(k == 0), stop=(k == KT - 1))

            def flush_group(mg, pst, rsv):
                # m-chunk mi covers cross rows 512*mg + 128*mi + [0,128):
                # upper 64 -> piece 2*mi, lower 64 -> piece 2*mi+1
                for mi in range(4):
                    ot = outp.tile([P, F], BF16, name=f"ot{mg}_{mi}",
                                   tag="ot")
                    for n in range(2):
                        nc.vector.tensor_copy(
                            ot[:, 512 * n: 512 * n + 512], pst[mi][n][:])
                    nc.sync.dma_start(rsv[2 * mi, 0:64, :], ot[0:64, :])
                    nc.sync.dma_start(rsv[2 * mi + 1, 0:64, :], ot[64:P, :])

            # ---- mg0: interleave x-block streaming, bf16 casts, stats
            #      accumulation, and the first half of the matmuls ----
            xl = []
            pst0 = psum_group(0)
            for k in range(KT):
                stg = stgp.tile([P, F], F32, name=f"stg{k}", tag="stg")
                nc.sync.dma_start(stg[:], xb[P * k: P * k + P, :])
                xlk = xlp.tile([P, F], BF16, name=f"xl{k}", tag=f"xl{k}")
                nc.vector.tensor_copy(xlk[:], stg[:])
                xl.append(xlk)
                xc_b = xc_load(0, k)
                mm_k(pst0, xc_b, xlk, k)
                if k == 0:
                    nc.vector.tensor_copy(sum_acc[:], stg[:])
                    nc.scalar.square(sq_acc[:], stg[:])
                else:
                    sq_t = sqp.tile([P, F], F32, tag="sqt")
                    nc.scalar.square(sq_t[:], stg[:])
                    nc.vector.tensor_add(sum_acc[:], sum_acc[:], stg[:])
                    nc.vector.tensor_add(sq_acc[:], sq_acc[:], sq_t[:])
                if k == 2:
                    # small constant loads, emitted late enough not to
                    # delay the first matmul's input casts
                    msk = p2p.tile([P, F], F32)
                    nc.gpsimd.dma_start(msk[:], dmask[:])
                    r08 = smallp.tile([8, F], F32)
                    nc.gpsimd.dma_start(r08[:],
                                        xb[0:1, :].to_broadcast((8, F)))
                    rT8 = smallp.tile([8, F], F32)
                    nc.gpsimd.dma_start(
                        rT8[:], xb[TS:TS + 1, :].to_broadcast((8, F)))
                    fx8 = smallp.tile([8, F], F32)
                    nc.gpsimd.dma_start(fx8[:],
                                        fixr[0:1, :].to_broadcast((8, F)))
                    ones8 = smallp.tile([P, 8], F32)
                    nc.vector.memset(ones8[:], 1.0)
                    zfillb = smallp.tile([8, F - 64], BF16)
                    nc.vector.memset(zfillb[:], 0.0)
            flush_group(0, pst0, rsvA)

            # ---- stats finalize: ones[128,8] matmul partition-reduces AND
            #      replicates the sums across 8 partitions; runs between
            #      the two matmul groups so it overlaps mg1 ----
            srow8 = smallp.tile([8, F], F32)
            qrow8 = smallp.tile([8, F], F32)
            for half in range(2):
                sl = slice(512 * half, 512 * half + 512)
                ps_a = psum.tile([8, 512], F32, name=f"psa{half}", tag="ps")
                nc.tensor.matmul(ps_a[:], ones8[:], sum_acc[:, sl],
                                 start=True, stop=True)
                nc.vector.tensor_copy(srow8[:, sl], ps_a[:])
                ps_b = psum.tile([8, 512], F32, name=f"psb{half}", tag="ps")
                nc.tensor.matmul(ps_b[:], ones8[:], sq_acc[:, sl],
                                 start=True, stop=True)
                nc.vector.tensor_copy(qrow8[:, sl], ps_b[:])

            # row math in fp32; final op of each chain writes bf16 for the
            # reduce-scatter buffers. S_c = srow - r0 + rT ;
            # q_c = qrow - r0^2 + rT^2 ; S_l = srow - fix ; q_l = qrow - fix^2
            tmp8 = smallp.tile([8, F], F32)
            tmp8b = smallp.tile([8, F], F32)
            s_c8 = smallp.tile([8, F], BF16)
            nc.vector.tensor_sub(tmp8[:], srow8[:], r08[:])
            nc.vector.tensor_add(s_c8[:], tmp8[:], rT8[:])
            q_c8 = smallp.tile([8, F], BF16)
            nc.scalar.square(tmp8[:], r08[:])
            nc.vector.tensor_sub(tmp8b[:], qrow8[:], tmp8[:])
            nc.scalar.square(tmp8[:], rT8[:])
            nc.vector.tensor_add(q_c8[:], tmp8b[:], tmp8[:])
            s_l8 = smallp.tile([8, F], BF16)
            nc.vector.tensor_sub(s_l8[:], srow8[:], fx8[:])
            q_l8 = smallp.tile([8, F], BF16)
            nc.scalar.square(tmp8[:], fx8[:])
            nc.vector.tensor_sub(q_l8[:], qrow8[:], tmp8[:])

            # stat writes (gpsimd queue):
            # RS_A rows 64/65: full S_l/q_l per piece (partition p -> piece)
            nc.gpsimd.dma_start(rsvA[:, 64, :], s_l8[:])
            nc.gpsimd.dma_start(rsvA[:, 65, :], q_l8[:])
            # chunk rows: S_c/q_c split into 16 chunks of 64; chunks 0..7
            # (= rows 0:512) go to RS_A row 66/67, chunks 8..15 to RS_B
            nc.gpsimd.dma_start(rsvA[:, 66, 0:64], s_c8[0:1, 0:512])
            nc.gpsimd.dma_start(rsvA[:, 67, 0:64], q_c8[0:1, 0:512])
            nc.gpsimd.dma_start(rsvB[:, 64, 0:64], s_c8[0:1, 512:F])
            nc.gpsimd.dma_start(rsvB[:, 65, 0:64], q_c8[0:1, 512:F])
            # zero filler for cols 64:F of the chunk rows
            nc.gpsimd.dma_start(rsvA[:, 66, 64:F], zfillb[:, 0:F - 64])
            nc.gpsimd.dma_start(rsvA[:, 67, 64:F], zfillb[:, 0:F - 64])
            nc.gpsimd.dma_start(rsvB[:, 64, 64:F], zfillb[:, 0:F - 64])
            nc.gpsimd.dma_start(rsvB[:, 65, 64:F], zfillb[:, 0:F - 64])

            # ---- RS_A: overlaps the second matmul group ----
            nc.gpsimd.collective_compute(
                "ReduceScatter", add,
                replica_groups=[list(range(NCORES))],
                ins=[rsA_in.opt()], outs=[rsA_out.opt()])

            # ---- mg1 ----
            pst1 = psum_group(1)
            for k in range(KT):
                xc_b = xc_load(1, k)
                mm_k(pst1, xc_b, xl[k], k)
            flush_group(1, pst1, rsvB)

            # ---- RS_B ----
            nc.gpsimd.collective_compute(
                "ReduceScatter", add,
                replica_groups=[list(range(NCORES))],
                ins=[rsB_in.opt()], outs=[rsB_out.opt()])

            # ---- phase 2: normalize this core's 2x64-row shard ----
            cr = p2p.tile([P, F], BF16)
            nc.sync.dma_start(cr[0:64, :], rsA_out[0:64, :])
            nc.sync.dma_start(cr[64:P, :], rsB_out[0:64, :])
            st = p2p.tile([4, F], BF16)          # matmul rhs: row 0 = S_l
            nc.scalar.dma_start(st[:], rsA_out[64:68, :])
            # partition-0 stat rows for ACT/DVE row math
            rsl = p2p.tile([4, F], BF16)
            nc.scalar.dma_start(rsl[0:1, :], rsA_out[64:65, :])
            rql = p2p.tile([4, F], BF16)
            nc.gpsimd.dma_start(rql[0:1, :], rsA_out[65:66, :])
            scin = p2p.tile([4, P], BF16)        # [chunkA | chunkB]
            nc.gpsimd.dma_start(scin[0:1, 0:64], rsA_out[66:67, 0:64])
            nc.gpsimd.dma_start(scin[0:1, 64:P], rsB_out[64:65, 0:64])
            qcin = p2p.tile([4, P], BF16)
            nc.scalar.dma_start(qcin[0:1, 0:64], rsA_out[67:68, 0:64])
            nc.scalar.dma_start(qcin[0:1, 64:P], rsB_out[65:66, 0:64])

            inv_n = 1.0 / NT
            rsqrt_n = float(np.sqrt(inv_n))

            # nl[j] = q_l[j] - S_l[j]^2/N  (row 0 of a zeroed [4, F] bf16
            # tile; rows 1-3 ride along as rhs rows so they must be finite)
            nl8 = p2p.tile([4, F], BF16)
            nc.vector.memset(nl8[:], 0.0)
            t_nl = p2p.tile([1, F], F32)
            nc.scalar.activation(t_nl[:], rsl[0:1, :], AF.Square,
                                 scale=rsqrt_n)
            nc.vector.tensor_sub(nl8[0:1, :], rql[0:1, :], t_nl[:])

            # scn row [1,128] = S_c/N ; ncol row [1,128] = q_c - S_c^2/N
            sc8 = p2p.tile([4, P], BF16)
            nc.vector.memset(sc8[:], 0.0)
            nc.scalar.mul(sc8[0:1, :], scin[0:1, :], inv_n)
            nc8 = p2p.tile([4, P], BF16)
            nc.vector.memset(nc8[:], 0.0)
            t_nc = p2p.tile([1, P], F32)
            nc.scalar.activation(t_nc[:], scin[0:1, :], AF.Square,
                                 scale=rsqrt_n)
            nc.vector.tensor_sub(nc8[0:1, :], qcin[0:1, :], t_nc[:])

            # cov = cr - outer(S_c/N, S_l); dsq = outer(ncol, nl)
            cov = p2p.tile([P, F], F32)
            work = p2p.tile([P, F], F32)
            for half in range(2):
                sl = slice(512 * half, 512 * half + 512)
                po = psum.tile([P, 512], F32, name=f"po{half}", tag="ps")
                nc.tensor.matmul(po[:], sc8[:], st[0:4, sl],
                                 start=True, stop=True)
                nc.vector.tensor_sub(cov[:, sl], cr[:, sl], po[:])
                pd = psum.tile([P, 512], F32, name=f"pd{half}", tag="ps")
                nc.tensor.matmul(pd[:], nc8[:], nl8[:, sl],
                                 start=True, stop=True)
                nc.vector.tensor_copy(work[:, sl], pd[:])

            # work = 1/sqrt(dsq) in one ACT op; |cov| * work * mask, reduced
            nc.scalar.activation(work[:], work[:], AF.Abs_reciprocal_sqrt)
            nc.vector.tensor_mul(work[:], work[:], cov[:])
            nc.vector.tensor_mul(work[:], work[:], msk[:])
            rsum = p2p.tile([P, 1], F32)
            nc.vector.tensor_reduce(rsum[:], work[:], mybir.AxisListType.X,
                                    add, apply_absolute_value=True)

            fin = psum.tile([1, 1], F32, name="fin", tag="ps")
            nc.tensor.matmul(fin[:], ones8[:, 0:1], rsum[:],
                             start=True, stop=True)
            fout = smallp.tile([1, 1], F32)
            nc.scalar.mul(fout[:], fin[:], 1.0 / (F * (F - 1.0)))
            nc.sync.dma_start(out[:], fout[:])

    nc.compile()
    return nc


def _in_maps(x: np.ndarray):
    x = np.ascontiguousarray(x, dtype=np.float32)
    maps = []
    for k in range(NCORES):
        lo = TS * k
        hi = min(lo + TS + 1, T)
        xb = np.zeros((TS + 1, F), dtype=np.float32)
        xb[: hi - lo] = x[lo:hi]
        fix = np.zeros((1, F), dtype=np.float32)
        if k == NCORES - 1:
            fix[0] = x[T - 1]
        # core k owns cross rows {64k + r} (r<64) and {512 + 64k + (r-64)}
        dmask = np.ones((P, F), dtype=np.float32)
        r = np.arange(P)
        gi = np.where(r < 64, 64 * k + r, 512 + 64 * k + (r - 64))
        dmask[r, gi] = 0.0
        maps.append({"xb": xb, "fix": fix, "dmask": dmask})
    return maps


def kernel(x: np.ndarray, _trace: bool = False, **_):
    if "nc" not in _CACHE:
        _CACHE["nc"] = _build()
    nc = _CACHE["nc"]
    res = run_bass_kernel_spmd(nc, _in_maps(x), core_ids=list(range(NCORES)),
                               trace=_trace)
    total = np.float32(0.0)
    for k in range(NCORES):
        total += np.float32(res.results[k]["out"][0, 0])
    _CACHE["last_results"] = res
    return np.asarray(total, dtype=np.float32)
